# revision 5
# baseline (speedup 1.0000x reference)
"""MissHitScatter (moe_routing) Trainium2 Bass kernel.

Reference semantics (PATH_NUM=4, IS_HIT=True):
    out = einsum('np,nd->pnd', one_hot(0, 4), inputs)   # [4, N, D]
i.e. out[0] = inputs, out[1:4] = 0.

Strategy: data-parallel shard of the token dim N=65536 across 8 cores
(8192 tokens/core). Per core the Bass program is a single DRAM->DRAM
DMA copy of the input shard into path slot 0 of the output. Paths 1..3
stay zero via the runtime's documented ExternalOutput pre-zeroing
contract (native run_bass_kernel_spmd pre-zeros output buffers before
run_neff; the axon/PJRT path donates zero-initialized buffers as the
outputs), so no zero-fill traffic is spent on them.
"""

import numpy as np

N_CORES = 8
N = 65536
D = 1024
P = 4
N_SHARD = N // N_CORES

_CACHE: dict = {}


def _build_nc():
    from concourse import bass
    import concourse.mybir as mybir

    nc = bass.Bass()
    x = nc.declare_dram_parameter("inputs", [N_SHARD, D], mybir.dt.float32, isOutput=False)
    out = nc.declare_dram_parameter("routed", [P, N_SHARD, D], mybir.dt.float32, isOutput=True)

    # Split the 32MB copy across all three DGE issue paths (SWDGE on gpsimd,
    # HWDGE on sync/SP and scalar/Activation). Shared-bus ceiling is
    # ~334 GB/s. Each queue ends with a small-descriptor (1KB, 45ns) tail
    # so a straggling engine drains its backlog in tiny quanta instead of
    # holding a 3us 64KB descriptor while 15 engines idle. Default Block
    # epilogue (passive semaphore barrier) — no_gpsimd_drain=True parks the
    # HWDGE engines in an active DRAIN that polls the queue dispatchers on
    # SDMA engine 15 and drags it ~15% below the pack for the whole run.
    Q = [  # (start_row, main_rows, tail_rows) per queue
        (0, 2512, 256),      # gpsimd
        (2768, 2576, 256),   # sync
        (5600, 2336, 256),   # scalar
    ]
    TAIL_LAST = 1024  # max descriptor BYTES for the tail dma (256 f32 elems)
    with (
        nc.Block() as block,
        nc.semaphore("dma_sem") as dma_sem,
    ):
        def issue(eng, qi):
            s, m, t = Q[qi]
            eng.dma_start(out=out[0, s:s + m], in_=x[s:s + m]).then_inc(dma_sem, 16)
            eng.dma_start(
                out=out[0, s + m:s + m + t], in_=x[s + m:s + m + t],
                max_dma_last_dim=TAIL_LAST,
            ).then_inc(dma_sem, 16)

        @block.sync
        def _(sp):
            issue(sp, 1)

        @block.scalar
        def _(act):
            issue(act, 2)

        @block.gpsimd
        def _(gp):
            issue(gp, 0)
            gp.wait_ge(dma_sem, 96)

    return nc


def _get_nc():
    if "nc" not in _CACHE:
        _CACHE["nc"] = _build_nc()
    return _CACHE["nc"]


def kernel(inputs: np.ndarray, **_run_kwargs) -> np.ndarray:
    from concourse.bass_utils import run_bass_kernel_spmd

    inputs = np.ascontiguousarray(inputs, dtype=np.float32)
    assert inputs.shape == (N, D), inputs.shape

    nc = _get_nc()
    shards = np.split(inputs, N_CORES, axis=0)
    in_maps = [{"inputs": s} for s in shards]
    res = run_bass_kernel_spmd(nc, in_maps, core_ids=list(range(N_CORES)), **_run_kwargs)
    _CACHE["last_results"] = res
    out = np.concatenate([r["routed"] for r in res.results], axis=1)
    # Paths 1..3 are structurally zero (one-hot on path 0). The device
    # readback already contains exact zeros there (pre-zeroed ExternalOutput
    # buffers, verified on HW); re-assert on the host so correctness never
    # hinges on that runtime detail.
    out[1:] = 0.0
    assert out.shape == (P, N, D)
    return out



# revision 6
# speedup vs baseline: 1.1641x; 1.1641x over previous
"""MissHitScatter (moe_routing) Trainium2 Bass kernel.

Reference semantics (PATH_NUM=4, IS_HIT=True):
    out = einsum('np,nd->pnd', one_hot(0, 4), inputs)   # [4, N, D]
i.e. out[0] = inputs, out[1:4] = 0.

Strategy: data-parallel shard of the token dim N=65536 across 8 cores
(8192 tokens/core). Per core the Bass program is a single DRAM->DRAM
DMA copy of the input shard into path slot 0 of the output. Paths 1..3
stay zero via the runtime's documented ExternalOutput pre-zeroing
contract (native run_bass_kernel_spmd pre-zeros output buffers before
run_neff; the axon/PJRT path donates zero-initialized buffers as the
outputs), so no zero-fill traffic is spent on them.
"""

import numpy as np

N_CORES = 8
N = 65536
D = 1024
P = 4
N_SHARD = N // N_CORES

_CACHE: dict = {}


def _build_nc():
    from concourse import bass
    import concourse.mybir as mybir

    nc = bass.Bass()
    x = nc.declare_dram_parameter("inputs", [N_SHARD, D], mybir.dt.float32, isOutput=False)
    out = nc.declare_dram_parameter("routed", [P, N_SHARD, D], mybir.dt.float32, isOutput=True)

    # Split the 32MB copy across all three DGE issue paths (SWDGE on gpsimd,
    # HWDGE on sync/SP and scalar/Activation). The shared DMA bus caps at
    # ~334 GB/s; three concurrent rings keep all 16 SDMA engines fed from
    # the end of the ~6us NEFF preamble. Keep total descriptor count low
    # (~490): every descriptor fetch taxes SDMA engine 15, which hosts the
    # three queue dispatchers (~6ns/desc of lost bandwidth-time there), so
    # high-descriptor-count variants straggle on that engine.
    R1, R2 = 2736, 5472  # gpsimd: rows [0,R1), sync: [R1,R2), scalar: [R2,8192)
    with (
        nc.Block() as block,
        nc.semaphore("dma_sem") as dma_sem,
    ):
        @block.sync
        def _(sp):
            sp.dma_start(out=out[0, R1:R2], in_=x[R1:R2]).then_inc(dma_sem, 16)

        @block.scalar
        def _(act):
            act.dma_start(out=out[0, R2:], in_=x[R2:]).then_inc(dma_sem, 16)

        @block.gpsimd
        def _(gp):
            gp.dma_start(out=out[0, :R1], in_=x[:R1]).then_inc(dma_sem, 16)
            gp.wait_ge(dma_sem, 48)

    return nc


def _get_nc():
    if "nc" not in _CACHE:
        _CACHE["nc"] = _build_nc()
    return _CACHE["nc"]


def kernel(inputs: np.ndarray, **_run_kwargs) -> np.ndarray:
    from concourse.bass_utils import run_bass_kernel_spmd

    inputs = np.ascontiguousarray(inputs, dtype=np.float32)
    assert inputs.shape == (N, D), inputs.shape

    nc = _get_nc()
    shards = np.split(inputs, N_CORES, axis=0)
    in_maps = [{"inputs": s} for s in shards]
    res = run_bass_kernel_spmd(nc, in_maps, core_ids=list(range(N_CORES)), **_run_kwargs)
    _CACHE["last_results"] = res
    out = np.concatenate([r["routed"] for r in res.results], axis=1)
    # Paths 1..3 are structurally zero (one-hot on path 0). The device
    # readback already contains exact zeros there (pre-zeroed ExternalOutput
    # buffers, verified on HW); re-assert on the host so correctness never
    # hinges on that runtime detail.
    out[1:] = 0.0
    assert out.shape == (P, N, D)
    return out

